# revision 45
# baseline (speedup 1.0000x reference)
"""GAT layer (nn_GAT_57543971832576) Bass/Tile kernel for 8 Trainium2 NeuronCores.

Math (reference):
    x' = x @ W + bias
    S_ij = leaky_relu(f1_i + f2_j, 0.2),  f1 = x'@phi1, f2 = x'@phi2
    A = softmax_j(where(adj+I > 0, S, -1e9))
    h = A @ x'

Reformulation used on device (core owns rows i = [c*1024, (c+1)*1024)):
    leaky_relu(s, 0.2) = 0.6*s + 0.4*|s|; softmax rows are invariant to the
    per-row shift 0.6*f1_i, so the masked attention numerator is
        B_ij = exp(0.4*|f1_i + f2_j| + 0.6*f2_j + Lp_ij)
    where Lp = 0 on edges (adj+I > 0) and -240 otherwise (additive log-space
    mask; exp underflows to exactly 0).  Using sum_j A_ij = 1 to pull the bias
    out of the values:
        P[k, i]  = sum_j x[j, k] * B^T[j, i]      (PE, lhsT = x tiles)
        den[i]   = sum_j B^T[j, i]                (PE, lhsT = ones; 24 of the
                                                   32 tile-pairs DVE-pair-summed)
        h[i, f]  = (P^T @ W)[i, f] / den_i + bias_f

    The Lp mask is folded in DURING the adjacency fetch for most tile-pairs:
    a gpsimd (SWDGE) accumulating DMA casts the fp8 mask to f16 and adds it
    onto the DVE-computed |s| tiles (CCE lines cap accumulating descriptors
    at 2048 elements, so these run at pair grain), making the per-tile
    elementwise chain just add -> abs -> (DMA +Lp) -> exp.  A few pairs
    (prologue + spread relief points) instead use a preloaded f16 mask and a
    DVE tensor add, which lets the exp stream start before the accum-DMA
    pipeline has spun up and relieves the SDMA engines mid-run.  Pairs 2-3's
    score prep is hoisted ahead of pair 0's consumers for the same reason.
    The diagonal self-loop is pre-ORed into the host-side mask, so no
    on-device correction terms are needed.

The host ships Lp pre-swizzled [p, t, i] so the device never transposes the
big matrix; the j-contraction runs directly over [j-part, i-free] tiles.
"""

import numpy as np

N = 8192
F = 128
NCORES = 8
R = N // NCORES          # rows per core (1024)
TJ = N // 128            # j-tiles (64)
ISUB = R // 128          # core-row subtiles (8)
CH = 8                   # f-projection chunks
TPC = TJ // CH           # j-tiles per chunk (8)
PSC = 0.015625           # 2^-6 epilogue rescale (f16 overflow guard)
DVE_PQS = (0, 1, 8, 14, 20, 26)  # mask via preloaded f16 + DVE add
NDVE = len(DVE_PQS)


def _pairsum(pq):
    # pairs whose den matmul is DVE pair-summed (PE/DVE balance; includes
    # the last pair so PE's post-exp tail is short)
    return pq % 2 == 0 or pq % 4 == 1 or pq == 31

_CACHE = {}
TRACE = False            # set True (e.g. from test.py) to capture an NTFF profile
LAST_EXEC_NS = None      # exec time from the last traced run
LAST_RESULTS = None      # full BassKernelResults of the last run


def _build_nc():
    import concourse.bass as bass
    import concourse.mybir as mybir
    import concourse.tile as tile

    f32 = mybir.dt.float32
    f16 = mybir.dt.float16
    i16 = mybir.dt.int16
    Alu = mybir.AluOpType
    Act = mybir.ActivationFunctionType

    nc = bass.Bass("TRN2", target_bir_lowering=False, debug=False,
                   num_devices=NCORES)

    f8 = mybir.dt.float8e4
    # log-space mask: most pairs fetch fp8 via the SWDGE accum-DMA
    # (cast+add onto |s| during the load); a few pairs (the prologue,
    # before the accum stream spins up, plus two mid-run relief points)
    # use a preloaded f16 mask + DVE tensor add instead
    lp8_d = nc.dram_tensor("lp8", [128, 2 * (TJ // 2 - NDVE), R], f8,
                           kind="ExternalInput").ap()
    lp16_d = nc.dram_tensor("lp16", [128, 2 * NDVE, R], f16,
                            kind="ExternalInput").ap()
    x16n_d = nc.dram_tensor("x16n", [128, TJ, F], f16, kind="ExternalInput").ap()
    xT16_d = nc.dram_tensor("xT16", [128, TJ, 128], f16, kind="ExternalInput").ap()
    xcT_d = nc.dram_tensor("xcT16", [128, ISUB, 128], f16, kind="ExternalInput").ap()
    W16_d = nc.dram_tensor("weight16", [128, 128], f16, kind="ExternalInput").ap()
    WT_d = nc.dram_tensor("weightT", [128, 128], f32, kind="ExternalInput").ap()
    bp_d = nc.dram_tensor("bp", [128, 4], f32, kind="ExternalInput").ap()
    br_d = nc.dram_tensor("biasrow", [1, 128], f32, kind="ExternalInput").ap()
    out_d = nc.dram_tensor("out", [R, F], f32, kind="ExternalOutput").ap()

    with tile.TileContext(nc) as tc:
        with tc.tile_pool(name="const", bufs=1) as cp, \
             tc.tile_pool(name="mmps", bufs=1, space="PSUM") as mmps, \
             tc.tile_pool(name="ppsA", bufs=1, space="PSUM") as ppsA, \
             tc.tile_pool(name="eps", bufs=2, space="PSUM") as eps:
            # ---------------- psum layout (8 banks) ----------------
            P0 = mmps.tile([128, 512], f32, name="P0")
            P1 = mmps.tile([128, 512], f32, name="P1")
            d0 = mmps.tile([1, 512], f32, name="d0")
            d1 = mmps.tile([1, 512], f32, name="d1")
            smA = ppsA.tile([128, 512], f32, name="smA")   # prep scratch
            smB = ppsA.tile([128, 160], f32, name="smB")   # f-projections + den col

            # ---------------- constants + inputs ----------------
            ones = cp.tile([1, 128], f32)
            nc.vector.memset(ones[:], 1.0)
            ones16 = cp.tile([128, 1], f16)
            nc.vector.memset(ones16[:], 1.0)
            ones16r = cp.tile([1, 128], f16)
            nc.vector.memset(ones16r[:], 1.0)
            # pre-warm the exp table load and the SWDGE (Q7) DMA path so
            # neither first-use cost lands on the first real exp / mask DMA
            warm = cp.tile([1, 128], f16)
            nc.scalar.activation(warm[0:1, 0:8], ones16r[0:1, 0:8],
                                 Act.Exp, scale=0.0)

            WT = cp.tile([128, 128], f32)
            nc.sync.dma_start(WT[:], WT_d)
            bp = cp.tile([128, 4], f32)
            nc.sync.dma_start(bp[:], bp_d)
            xcT = cp.tile([128, ISUB, 128], f16)
            nc.sync.dma_start(xcT[:], xcT_d)
            xT16 = cp.tile([128, TJ, 128], f16)
            x16 = cp.tile([128, TJ, F], f16)
            br = cp.tile([1, 128], f32)
            W16 = cp.tile([128, 128], f16)

            # ---------------- prep ----------------
            # Only WT/bp/xcT are in flight here: every DMA emitted before a
            # compute op can end up on its wait path (pooled completion
            # lanes), so the big x loads are emitted after the f1b prep.
            # Wphi = W @ phi (host supplies W^T), so f = x @ Wphi
            nc.tensor.matmul(smA[:, 128:130], WT[:], bp[:, 1:3],
                             start=True, stop=True)
            Wphi16 = cp.tile([128, 2], f16)
            nc.vector.tensor_copy(Wphi16[:], smA[:, 128:130])

            # core-row f1 projections: f1row = phi1^T @ Xcore^T (f16 rhs so
            # the broadcast matmul runs at f16 rate; c12 is folded into f2c)
            f1row = cp.tile([1, R], f16)
            f1b = cp.tile([128, R], f16)
            for g in range(2):
                fr = eps.tile([128, 512], f32, tag="fps")
                nc.tensor.matmul(
                    fr[0:1, :], Wphi16[:, 0:1],
                    xcT[:, 4 * g:4 * g + 4, :].rearrange("p a b -> p (a b)"),
                    start=True, stop=True)
                if g == 0:
                    nc.scalar.copy(f1row[0:1, 0:512], fr[0:1, :])
                else:
                    nc.vector.tensor_copy(f1row[0:1, 512:1024], fr[0:1, :])
            for g in range(2):
                fb = eps.tile([128, 512], f32, tag="fps")
                nc.tensor.matmul(fb[:], ones16r[:],
                                 f1row[0:1, 512 * g:512 * (g + 1)],
                                 start=True, stop=True)
                nc.vector.tensor_copy(f1b[:, 512 * g:512 * (g + 1)], fb[:])

            # big x loads + masks, emitted only now (see note above)
            nc.sync.dma_start(xT16[:, 0:TPC, :], xT16_d[:, 0:TPC, :])
            nc.sync.dma_start(x16[:, 0:2 * TPC, :], x16n_d[:, 0:2 * TPC, :])
            nc.sync.dma_start(xT16[:, TPC:3 * TPC, :], xT16_d[:, TPC:3 * TPC, :])
            nc.sync.dma_start(br[:], br_d)
            nc.sync.dma_start(W16[:], W16_d)
            # c1 = b@phi1, c2 = b@phi2 (bias-fold constants), broadcasts
            nc.tensor.matmul(smA[0:1, 132:134], bp[:, 0:1], bp[:, 1:3],
                             start=True, stop=True)
            crow = cp.tile([1, 2], f32)
            nc.scalar.copy(crow[:], smA[0:1, 132:134])
            c12 = cp.tile([1, 1], f32)
            nc.vector.tensor_tensor(c12[:], crow[0:1, 0:1], crow[0:1, 1:2],
                                    op=Alu.add)
            nc.tensor.matmul(smA[:, 136:137], ones[:], c12[:],
                             start=True, stop=True)
            c12b = cp.tile([128, 1], f32)
            nc.scalar.copy(c12b[:], smA[:, 136:137])
            nc.tensor.matmul(smA[:, 140:141], ones[:], crow[0:1, 1:2],
                             start=True, stop=True)
            c2b06 = cp.tile([128, 1], f32)
            nc.scalar.activation(c2b06[:], smA[:, 140:141], Act.Copy, scale=0.6)


            f2c = cp.tile([128, TJ], f32)
            wb = cp.tile([128, TJ], f32)    # ACT bias: 0.6*f2_j (= log w_j)

            def emit_fcols(ch):
                # chunk ch's f2 projections (column form) + exp-bias scalars
                c0 = 16 * ch
                for tt in range(TPC):
                    t = ch * TPC + tt
                    nc.tensor.matmul(smB[:, 2 * t:2 * t + 2],
                                     xT16[:, t, :], Wphi16[:],
                                     start=True, stop=True)
                nc.vector.tensor_scalar(f2c[:, ch * TPC:(ch + 1) * TPC],
                                        smB[:, c0 + 1:c0 + 16:2],
                                        c12b[:], None, op0=Alu.add)
                nc.vector.tensor_scalar(wb[:, ch * TPC:(ch + 1) * TPC],
                                        smB[:, c0 + 1:c0 + 16:2],
                                        0.6, c2b06[:],
                                        op0=Alu.mult, op1=Alu.add)

            # ---------------- main loop (32 tile-pairs) ----------------
            with tc.tile_pool(name="sp", bufs=8) as spp, \
                 tc.tile_pool(name="gp", bufs=4) as gpp, \
                 tc.tile_pool(name="lpp", bufs=3) as lpp, \
                 tc.tile_pool(name="b2p", bufs=2) as b2p:
                emit_fcols(0)
                emit_fcols(1)
                # trailing big x slices: second HWDGE ring (ACT queue, idle
                # this early), and emitted AFTER the prep section -- DMA
                # completion semaphores are pooled into 8 lanes, so anything
                # emitted before the prep matmuls would be transitively
                # waited on by them (costs ~15us of prologue)
                nc.scalar.dma_start(xT16[:, 3 * TPC:TJ, :],
                                    xT16_d[:, 3 * TPC:TJ, :])
                nc.scalar.dma_start(x16[:, 2 * TPC:TJ, :],
                                    x16n_d[:, 2 * TPC:TJ, :])
                # f16 masks for the DVE-add pairs, loaded up front (small)
                lp16 = cp.tile([128, 2 * NDVE, R], f16)
                nc.sync.dma_start(lp16[:], lp16_d[:])

                sptiles = {}

                def emit_score(pq):
                    # |s| tiles + mask for pair pq (DVE-add for the first
                    # two pairs; SWDGE accum-DMA otherwise)
                    t0 = 2 * pq
                    spair = spp.tile([128, 2, R], f16, tag="sp")
                    sptiles[pq] = spair
                    for k in range(2):
                        nc.vector.tensor_scalar(
                            spair[:, k, :], f1b[:],
                            f2c[:, t0 + k:t0 + k + 1], None, op0=Alu.add)
                    nc.vector.tensor_scalar(
                        spair[:].rearrange("p a b -> p (a b)").bitcast(i16),
                        spair[:].rearrange("p a b -> p (a b)").bitcast(i16),
                        0x7FFF, None, op0=Alu.bitwise_and)
                    if pq in DVE_PQS:
                        sl = 2 * DVE_PQS.index(pq)
                        nc.vector.tensor_tensor(
                            spair[:].rearrange("p a b -> p (a b)"),
                            spair[:].rearrange("p a b -> p (a b)"),
                            lp16[:, sl:sl + 2, :].rearrange("p a b -> p (a b)"),
                            op=Alu.add)
                    else:
                        na = pq - sum(1 for q in DVE_PQS if q < pq)
                        nc.gpsimd.dma_start(spair[:],
                                            lp8_d[:, 2 * na:2 * na + 2, :],
                                            accum_op=Alu.add)

                # hoist the first four pairs' score prep: pair 0 first so
                # exp0 fires as early as possible, pairs 2-3 so the
                # accum-DMA pipeline spins up under the first DVE exps
                emit_score(0)
                emit_score(1)
                emit_score(2)
                emit_score(3)
                for pq in range(TJ // 2):
                    t0 = 2 * pq
                    if t0 % TPC == 0 and t0 // TPC + 2 < CH:
                        emit_fcols(t0 // TPC + 2)
                    if pq in sptiles:
                        spair = sptiles.pop(pq)
                    else:
                        emit_score(pq)
                        spair = sptiles.pop(pq)
                    gpair = gpp.tile([128, 2, R], f16, tag="g")
                    for k in range(2):
                        nc.scalar.activation(
                            gpair[:, k, :], spair[:, k, :], Act.Exp,
                            scale=0.4, bias=wb[:, t0 + k:t0 + k + 1])
                    for k in range(2):
                        t = t0 + k
                        nc.tensor.matmul(P0[:], x16[:, t, :],
                                         gpair[:, k, 0:512],
                                         start=(t == 0), stop=(t == TJ - 1))
                        nc.tensor.matmul(P1[:], x16[:, t, :],
                                         gpair[:, k, 512:1024],
                                         start=(t == 0), stop=(t == TJ - 1))
                    # den: DVE pair-sum for a spread subset (PE/DVE balance)
                    if _pairsum(pq):
                        b2 = b2p.tile([128, R], f16, tag="b2")
                        nc.vector.tensor_tensor(
                            b2[:], gpair[:, 0, :], gpair[:, 1, :], op=Alu.add)
                        nc.tensor.matmul(d0[:], ones16[:], b2[:, 0:512],
                                         start=(pq == 0),
                                         stop=(pq == TJ // 2 - 1))
                        nc.tensor.matmul(d1[:], ones16[:], b2[:, 512:1024],
                                         start=(pq == 0),
                                         stop=(pq == TJ // 2 - 1))
                    else:
                        for k in range(2):
                            first = t0 + k == 0
                            last = t0 + k == TJ - 1
                            nc.tensor.matmul(d0[:], ones16[:],
                                             gpair[:, k, 0:512],
                                             start=first, stop=last)
                            nc.tensor.matmul(d1[:], ones16[:],
                                             gpair[:, k, 512:1024],
                                             start=first, stop=last)

            # bias broadcast tile (emitted late: keeps the early DVE queue
            # clear; only the epilogue consumes it)
            nc.tensor.matmul(smA[:, 256:384], ones[:], br[:],
                             start=True, stop=True)
            biasb = cp.tile([128, 128], f32)
            nc.vector.tensor_copy(biasb[:], smA[:, 256:384])
            # ---------------- epilogue ----------------
            # h[i,f] = (P^T @ W)[i,f] / den_i + bias_f, subtile-pipelined.
            with tc.tile_pool(name="ep", bufs=1) as ep:
                Pc16 = ep.tile([128, R], f16)
                for k in range(ISUB):
                    Pps = P0 if k < 4 else P1
                    nc.vector.tensor_scalar(
                        Pc16[:, 128 * k:128 * (k + 1)],
                        Pps[:, 128 * (k % 4):128 * (k % 4) + 128],
                        PSC, None, op0=Alu.mult)
                dsb0 = ep.tile([1, 512], f32)
                dsb1 = ep.tile([1, 512], f32)
                nc.scalar.copy(dsb0[:], d0[:])
                nc.scalar.copy(dsb1[:], d1[:])
                # den row -> column form [128, ISUB] via 1-wide transposes
                for k in range(ISUB):
                    dsb = dsb0 if k < 4 else dsb1
                    nc.tensor.matmul(
                        smB[:, 144 + k:145 + k],
                        dsb[0:1, 128 * (k % 4):128 * (k % 4) + 128],
                        ones[0:1, 0:1], start=True, stop=True)
                dcs = ep.tile([128, ISUB], f32)
                nc.vector.tensor_scalar(dcs[:], smB[:, 144:144 + ISUB], PSC,
                                        None, op0=Alu.mult)
                recb = ep.tile([128, ISUB], f32)
                nc.vector.reciprocal(recb[:], dcs[:])
                hout = ep.tile([128, ISUB, 128], f32)
                for k in range(ISUB):
                    hps = smA[:, 128 * (k % 4):128 * (k % 4) + 128]
                    nc.tensor.matmul(hps, Pc16[:, 128 * k:128 * (k + 1)],
                                     W16[:], start=True, stop=True)
                    nc.vector.scalar_tensor_tensor(
                        hout[:, k, :], hps, recb[:, k:k + 1], biasb[:],
                        op0=Alu.mult, op1=Alu.add)
                    if k % 2 == 1:
                        nc.sync.dma_start(
                            out_d.rearrange("(a p) f -> p a f", p=128)[:, k - 1:k + 1, :],
                            hout[:, k - 1:k + 1, :])

    # Walrus fits at most one sync-wait per instruction; Tile emits more.
    # Run bacc's splitter (extra waits move onto EventSemaphore insts).
    from concourse.bass import _bass_rust
    _bass_rust.generate_event_semaphores(nc)
    return nc


def kernel(adj, input, weight, bias, phi):
    """Full inputs in, full output out. Shards row-wise across 8 NeuronCores."""
    adj = np.ascontiguousarray(np.asarray(adj, dtype=np.float32))
    x = np.ascontiguousarray(np.asarray(input, dtype=np.float32))
    W = np.ascontiguousarray(np.asarray(weight, dtype=np.float32))
    b = np.ascontiguousarray(np.asarray(bias, dtype=np.float32))
    phi = np.ascontiguousarray(np.asarray(phi, dtype=np.float32))

    if not _CACHE.get("use_fallback"):
        try:
            return _kernel_bass(adj, x, W, b, phi)
        except Exception:
            import traceback
            traceback.print_exc()
            _CACHE["use_fallback"] = True
    return _kernel_jax_fallback(adj, x, W, b, phi)


def _kernel_bass(adj, x, W, b, phi):
    from concourse.bass_utils import run_bass_kernel_spmd

    if "nc" not in _CACHE:
        _CACHE["nc"] = _build_nc()
    nc = _CACHE["nc"]

    # lp[c, p, t, il] = 0 if edge/diag at (row c*R+il, col t*128+p) else -240
    # (transposed + swizzled log-space mask, f8e4m3: 0x00 / 0xF7 = -240; the
    # SWDGE accum-DMA casts to f16 while adding onto |s|)
    mask = adj.reshape(NCORES, R, TJ, 128).transpose(0, 3, 2, 1) > 0
    iloc = np.arange(R)
    for c in range(NCORES):
        gi = c * R + iloc                       # global row index
        mask[c, gi % 128, gi // 128, iloc] = True   # self-loop
    acc_tiles = [t for pq in range(TJ // 2) if pq not in DVE_PQS
                 for t in (2 * pq, 2 * pq + 1)]
    dve_tiles = [t for pq in DVE_PQS for t in (2 * pq, 2 * pq + 1)]
    lp8 = np.where(mask[:, :, acc_tiles, :],
                   np.uint8(0x00), np.uint8(0xF7))    # f8e4m3: 0 / -240
    lp16 = np.where(mask[:, :, dve_tiles, :],
                    np.float16(0), np.float16(-240))
    xT = np.ascontiguousarray(x.T).astype(np.float16)          # [128, 8192]
    x16n = np.ascontiguousarray(
        x.reshape(TJ, 128, F).transpose(1, 0, 2)).astype(np.float16)
    bp = np.ascontiguousarray(
        np.stack([b, phi[:F, 0], phi[F:, 0], np.zeros_like(b)], axis=1)
    ).astype(np.float32)

    in_maps = []
    for c in range(NCORES):
        r0 = c * R
        in_maps.append({
            "weightT": np.ascontiguousarray(W.T),
            "weight16": np.ascontiguousarray(W.astype(np.float16)),
            "biasrow": np.ascontiguousarray(b.reshape(1, F)),
            "lp8": np.ascontiguousarray(lp8[c]),
            "lp16": np.ascontiguousarray(lp16[c]),
            "x16n": x16n,
            "xT16": xT.reshape(128, TJ, 128),
            "xcT16": np.ascontiguousarray(
                xT[:, r0:r0 + R]).reshape(128, ISUB, 128),
            "bp": bp,
        })

    res = run_bass_kernel_spmd(nc, in_maps, core_ids=list(range(NCORES)),
                               trace=TRACE)
    global LAST_EXEC_NS, LAST_RESULTS
    LAST_RESULTS = res
    LAST_EXEC_NS = res.exec_time_ns
    parts = [res.results[c]["out"] for c in range(NCORES)]
    return np.concatenate(parts, axis=0).astype(np.float32)


def _kernel_jax_fallback(adj, x, W, b, phi):
    """Device fallback (sharded jax on the 8 NeuronCores) if the Bass path
    fails to compile/run in this environment."""
    import jax
    import jax.numpy as jnp
    from jax import lax
    from jax.sharding import Mesh, PartitionSpec, NamedSharding

    devs = jax.devices()[:NCORES]
    mesh = Mesh(np.asarray(devs), ("i",))
    row = NamedSharding(mesh, PartitionSpec("i", None))
    rep = NamedSharding(mesh, PartitionSpec())

    @jax.jit
    def f(adj_s, x_r, W_r, b_r, phi_r):
        xp = x_r @ W_r + b_r
        f1 = xp @ phi_r[:F]                      # [N, 1]
        f2 = xp @ phi_r[F:]                      # [N, 1]
        w = jnp.exp(jnp.float32(0.6) * f2[:, 0])  # [N]
        ri = lax.broadcasted_iota(jnp.int32, (N, N), 0)
        ci = lax.broadcasted_iota(jnp.int32, (N, N), 1)
        m = (adj_s > 0) | (ri == ci)
        G = jnp.exp(jnp.float32(0.4) * jnp.abs(f1 + f2.T))
        B = jnp.where(m, G * w[None, :], jnp.float32(0.0)).astype(jnp.float16)
        xpa = jnp.concatenate([xp, jnp.ones((N, 1), jnp.float32)],
                              axis=1).astype(jnp.float16)
        num = (B @ xpa).astype(jnp.float32)      # [N/8, F+1]
        return num[:, :F] / num[:, F:F + 1]

    args = (jax.device_put(adj, row), jax.device_put(x, rep),
            jax.device_put(W, rep), jax.device_put(b, rep),
            jax.device_put(phi, rep))
    out = f(*args)
    out.block_until_ready()
    if TRACE:
        import time
        global LAST_EXEC_NS
        reps = 5
        t0 = time.perf_counter()
        for _ in range(reps):
            out = f(*args)
        out.block_until_ready()
        LAST_EXEC_NS = int((time.perf_counter() - t0) / reps * 1e9)
    return np.asarray(out).astype(np.float32)


# revision 46
# speedup vs baseline: 1.1125x; 1.1125x over previous
"""GAT layer (nn_GAT_57543971832576) Bass/Tile kernel for 8 Trainium2 NeuronCores.

Math (reference):
    x' = x @ W + bias
    S_ij = leaky_relu(f1_i + f2_j, 0.2),  f1 = x'@phi1, f2 = x'@phi2
    A = softmax_j(where(adj+I > 0, S, -1e9))
    h = A @ x'

Reformulation used on device (core owns rows i = [c*1024, (c+1)*1024)):
    leaky_relu(s, 0.2) = 0.6*s + 0.4*|s|; softmax rows are invariant to the
    per-row shift 0.6*f1_i, so the masked attention numerator is
        B_ij = exp(0.4*|f1_i + f2_j| + 0.6*f2_j + Lp_ij)
    where Lp = 0 on edges (adj+I > 0) and -240 otherwise (additive log-space
    mask; exp underflows to exactly 0).  Using sum_j A_ij = 1 to pull the bias
    out of the values:
        P[k, i]  = sum_j x[j, k] * B^T[j, i]      (PE, lhsT = x tiles)
        den[i]   = sum_j B^T[j, i]                (PE, lhsT = ones; 24 of the
                                                   32 tile-pairs DVE-pair-summed)
        h[i, f]  = (P^T @ W)[i, f] / den_i + bias_f

    The Lp mask is folded in DURING the adjacency fetch for most tile-pairs:
    a gpsimd (SWDGE) accumulating DMA casts the fp8 mask to f16 and adds it
    onto the DVE-computed |s| tiles (CCE lines cap accumulating descriptors
    at 2048 elements, so these run at pair grain), making the per-tile
    elementwise chain just add -> abs -> (DMA +Lp) -> exp.  A few pairs
    (prologue + spread relief points) instead use a preloaded f16 mask and a
    DVE tensor add, which lets the exp stream start before the accum-DMA
    pipeline has spun up and relieves the SDMA engines mid-run.  Pairs 2-3's
    score prep is hoisted ahead of pair 0's consumers for the same reason.
    The diagonal self-loop is pre-ORed into the host-side mask, so no
    on-device correction terms are needed.

The host ships Lp pre-swizzled [p, t, i] so the device never transposes the
big matrix; the j-contraction runs directly over [j-part, i-free] tiles.
"""

import numpy as np

N = 8192
F = 128
NCORES = 8
R = N // NCORES          # rows per core (1024)
TJ = N // 128            # j-tiles (64)
ISUB = R // 128          # core-row subtiles (8)
CH = 8                   # f-projection chunks
TPC = TJ // CH           # j-tiles per chunk (8)
PSC = 0.015625           # 2^-6 epilogue rescale (f16 overflow guard)
DVE_PQS = (0, 1, 8, 14, 20, 26)  # mask via preloaded f16 + DVE add
NDVE = len(DVE_PQS)


def _pairsum(pq):
    # pairs whose den matmul is DVE pair-summed (PE/DVE balance; the very
    # last pair full-width so PE finishes right after the final exp)
    return pq % 2 == 0 or pq % 4 == 1

_CACHE = {}
TRACE = False            # set True (e.g. from test.py) to capture an NTFF profile
LAST_EXEC_NS = None      # exec time from the last traced run
LAST_RESULTS = None      # full BassKernelResults of the last run


def _build_nc():
    import concourse.bass as bass
    import concourse.mybir as mybir
    import concourse.tile as tile

    f32 = mybir.dt.float32
    f16 = mybir.dt.float16
    i16 = mybir.dt.int16
    Alu = mybir.AluOpType
    Act = mybir.ActivationFunctionType

    nc = bass.Bass("TRN2", target_bir_lowering=False, debug=False,
                   num_devices=NCORES)

    f8 = mybir.dt.float8e4
    # log-space mask: most pairs fetch fp8 via the SWDGE accum-DMA
    # (cast+add onto |s| during the load); a few pairs (the prologue,
    # before the accum stream spins up, plus two mid-run relief points)
    # use a preloaded f16 mask + DVE tensor add instead
    lp8_d = nc.dram_tensor("lp8", [128, 2 * (TJ // 2 - NDVE), R], f8,
                           kind="ExternalInput").ap()
    lp16_d = nc.dram_tensor("lp16", [128, 2 * NDVE, R], f16,
                            kind="ExternalInput").ap()
    x16n_d = nc.dram_tensor("x16n", [128, TJ, F], f16, kind="ExternalInput").ap()
    xT16_d = nc.dram_tensor("xT16", [128, TJ, 128], f16, kind="ExternalInput").ap()
    xcT_d = nc.dram_tensor("xcT16", [128, ISUB, 128], f16, kind="ExternalInput").ap()
    W16_d = nc.dram_tensor("weight16", [128, 128], f16, kind="ExternalInput").ap()
    WT_d = nc.dram_tensor("weightT", [128, 128], f32, kind="ExternalInput").ap()
    bp_d = nc.dram_tensor("bp", [128, 4], f32, kind="ExternalInput").ap()
    br_d = nc.dram_tensor("biasrow", [1, 128], f32, kind="ExternalInput").ap()
    out_d = nc.dram_tensor("out", [R, F], f32, kind="ExternalOutput").ap()

    with tile.TileContext(nc) as tc:
        with tc.tile_pool(name="const", bufs=1) as cp, \
             tc.tile_pool(name="mmps", bufs=1, space="PSUM") as mmps, \
             tc.tile_pool(name="ppsA", bufs=1, space="PSUM") as ppsA, \
             tc.tile_pool(name="eps", bufs=2, space="PSUM") as eps:
            # ---------------- psum layout (8 banks) ----------------
            P0 = mmps.tile([128, 512], f32, name="P0")
            P1 = mmps.tile([128, 512], f32, name="P1")
            d0 = mmps.tile([1, 512], f32, name="d0")
            d1 = mmps.tile([1, 512], f32, name="d1")
            smA = ppsA.tile([128, 512], f32, name="smA")   # prep scratch
            smB = ppsA.tile([128, 160], f32, name="smB")   # f-projections + den col

            # ---------------- constants + inputs ----------------
            ones = cp.tile([1, 128], f32)
            nc.vector.memset(ones[:], 1.0)
            ones16 = cp.tile([128, 1], f16)
            nc.vector.memset(ones16[:], 1.0)
            ones16r = cp.tile([1, 128], f16)
            nc.vector.memset(ones16r[:], 1.0)
            # pre-warm the exp table load and the SWDGE (Q7) DMA path so
            # neither first-use cost lands on the first real exp / mask DMA
            warm = cp.tile([1, 128], f16)
            nc.scalar.activation(warm[0:1, 0:8], ones16r[0:1, 0:8],
                                 Act.Exp, scale=0.0)

            WT = cp.tile([128, 128], f32)
            nc.sync.dma_start(WT[:], WT_d)
            bp = cp.tile([128, 4], f32)
            nc.sync.dma_start(bp[:], bp_d)
            xcT = cp.tile([128, ISUB, 128], f16)
            nc.sync.dma_start(xcT[:], xcT_d)
            xT16 = cp.tile([128, TJ, 128], f16)
            x16 = cp.tile([128, TJ, F], f16)
            br = cp.tile([1, 128], f32)
            W16 = cp.tile([128, 128], f16)

            # ---------------- prep ----------------
            # Only WT/bp/xcT are in flight here: every DMA emitted before a
            # compute op can end up on its wait path (pooled completion
            # lanes), so the big x loads are emitted after the f1b prep.
            # Wphi = W @ phi (host supplies W^T), so f = x @ Wphi
            nc.tensor.matmul(smA[:, 128:130], WT[:], bp[:, 1:3],
                             start=True, stop=True)
            Wphi16 = cp.tile([128, 2], f16)
            nc.vector.tensor_copy(Wphi16[:], smA[:, 128:130])

            # core-row f1 projections: f1row = phi1^T @ Xcore^T (f16 rhs so
            # the broadcast matmul runs at f16 rate; c12 is folded into f2c)
            f1row = cp.tile([1, R], f16)
            f1b = cp.tile([128, R], f16)
            for g in range(2):
                fr = eps.tile([128, 512], f32, tag="fps")
                nc.tensor.matmul(
                    fr[0:1, :], Wphi16[:, 0:1],
                    xcT[:, 4 * g:4 * g + 4, :].rearrange("p a b -> p (a b)"),
                    start=True, stop=True)
                if g == 0:
                    nc.scalar.copy(f1row[0:1, 0:512], fr[0:1, :])
                else:
                    nc.vector.tensor_copy(f1row[0:1, 512:1024], fr[0:1, :])
            for g in range(2):
                fb = eps.tile([128, 512], f32, tag="fps")
                nc.tensor.matmul(fb[:], ones16r[:],
                                 f1row[0:1, 512 * g:512 * (g + 1)],
                                 start=True, stop=True)
                nc.vector.tensor_copy(f1b[:, 512 * g:512 * (g + 1)], fb[:])

            # big x loads + masks, emitted only now (see note above)
            nc.sync.dma_start(xT16[:, 0:TPC, :], xT16_d[:, 0:TPC, :])
            nc.sync.dma_start(x16[:, 0:2 * TPC, :], x16n_d[:, 0:2 * TPC, :])
            nc.sync.dma_start(xT16[:, TPC:3 * TPC, :], xT16_d[:, TPC:3 * TPC, :])
            nc.sync.dma_start(br[:], br_d)
            nc.sync.dma_start(W16[:], W16_d)
            # c1 = b@phi1, c2 = b@phi2 (bias-fold constants), broadcasts
            nc.tensor.matmul(smA[0:1, 132:134], bp[:, 0:1], bp[:, 1:3],
                             start=True, stop=True)
            crow = cp.tile([1, 2], f32)
            nc.scalar.copy(crow[:], smA[0:1, 132:134])
            c12 = cp.tile([1, 1], f32)
            nc.vector.tensor_tensor(c12[:], crow[0:1, 0:1], crow[0:1, 1:2],
                                    op=Alu.add)
            nc.tensor.matmul(smA[:, 136:137], ones[:], c12[:],
                             start=True, stop=True)
            c12b = cp.tile([128, 1], f32)
            nc.scalar.copy(c12b[:], smA[:, 136:137])
            nc.tensor.matmul(smA[:, 140:141], ones[:], crow[0:1, 1:2],
                             start=True, stop=True)
            c2b06 = cp.tile([128, 1], f32)
            nc.scalar.activation(c2b06[:], smA[:, 140:141], Act.Copy, scale=0.6)


            f2c = cp.tile([128, TJ], f32)
            wb = cp.tile([128, TJ], f32)    # ACT bias: 0.6*f2_j (= log w_j)

            def emit_fcols(ch):
                # chunk ch's f2 projections (column form) + exp-bias scalars
                c0 = 16 * ch
                for tt in range(TPC):
                    t = ch * TPC + tt
                    nc.tensor.matmul(smB[:, 2 * t:2 * t + 2],
                                     xT16[:, t, :], Wphi16[:],
                                     start=True, stop=True)
                nc.vector.tensor_scalar(f2c[:, ch * TPC:(ch + 1) * TPC],
                                        smB[:, c0 + 1:c0 + 16:2],
                                        c12b[:], None, op0=Alu.add)
                nc.vector.tensor_scalar(wb[:, ch * TPC:(ch + 1) * TPC],
                                        smB[:, c0 + 1:c0 + 16:2],
                                        0.6, c2b06[:],
                                        op0=Alu.mult, op1=Alu.add)

            # ---------------- main loop (32 tile-pairs) ----------------
            with tc.tile_pool(name="sp", bufs=8) as spp, \
                 tc.tile_pool(name="gp", bufs=4) as gpp, \
                 tc.tile_pool(name="lpp", bufs=3) as lpp, \
                 tc.tile_pool(name="b2p", bufs=2) as b2p:
                emit_fcols(0)
                emit_fcols(1)
                # trailing big x slices: second HWDGE ring (ACT queue, idle
                # this early), and emitted AFTER the prep section -- DMA
                # completion semaphores are pooled into 8 lanes, so anything
                # emitted before the prep matmuls would be transitively
                # waited on by them (costs ~15us of prologue)
                nc.scalar.dma_start(xT16[:, 3 * TPC:TJ, :],
                                    xT16_d[:, 3 * TPC:TJ, :])
                nc.scalar.dma_start(x16[:, 2 * TPC:TJ, :],
                                    x16n_d[:, 2 * TPC:TJ, :])
                # f16 masks for the DVE-add pairs, loaded up front (small)
                lp16 = cp.tile([128, 2 * NDVE, R], f16)
                nc.sync.dma_start(lp16[:], lp16_d[:])

                sptiles = {}

                def emit_score(pq):
                    # |s| tiles + mask for pair pq (DVE-add for the first
                    # two pairs; SWDGE accum-DMA otherwise)
                    t0 = 2 * pq
                    spair = spp.tile([128, 2, R], f16, tag="sp")
                    sptiles[pq] = spair
                    for k in range(2):
                        nc.vector.tensor_scalar(
                            spair[:, k, :], f1b[:],
                            f2c[:, t0 + k:t0 + k + 1], None, op0=Alu.add)
                    nc.vector.tensor_scalar(
                        spair[:].rearrange("p a b -> p (a b)").bitcast(i16),
                        spair[:].rearrange("p a b -> p (a b)").bitcast(i16),
                        0x7FFF, None, op0=Alu.bitwise_and)
                    if pq in DVE_PQS:
                        sl = 2 * DVE_PQS.index(pq)
                        nc.vector.tensor_tensor(
                            spair[:].rearrange("p a b -> p (a b)"),
                            spair[:].rearrange("p a b -> p (a b)"),
                            lp16[:, sl:sl + 2, :].rearrange("p a b -> p (a b)"),
                            op=Alu.add)
                    else:
                        na = pq - sum(1 for q in DVE_PQS if q < pq)
                        nc.gpsimd.dma_start(spair[:],
                                            lp8_d[:, 2 * na:2 * na + 2, :],
                                            accum_op=Alu.add)

                # hoist pairs 2-3's score prep ahead of pair 0's consumers
                # so the accum-DMA pipeline spins up under the first DVE exps
                emit_score(2)
                emit_score(3)
                for pq in range(TJ // 2):
                    t0 = 2 * pq
                    if t0 % TPC == 0 and t0 // TPC + 2 < CH:
                        emit_fcols(t0 // TPC + 2)
                    if pq in sptiles:
                        spair = sptiles.pop(pq)
                    else:
                        emit_score(pq)
                        spair = sptiles.pop(pq)
                    gpair = gpp.tile([128, 2, R], f16, tag="g")
                    for k in range(2):
                        nc.scalar.activation(
                            gpair[:, k, :], spair[:, k, :], Act.Exp,
                            scale=0.4, bias=wb[:, t0 + k:t0 + k + 1])
                    for k in range(2):
                        t = t0 + k
                        nc.tensor.matmul(P0[:], x16[:, t, :],
                                         gpair[:, k, 0:512],
                                         start=(t == 0), stop=(t == TJ - 1))
                        nc.tensor.matmul(P1[:], x16[:, t, :],
                                         gpair[:, k, 512:1024],
                                         start=(t == 0), stop=(t == TJ - 1))
                    # den: DVE pair-sum for a spread subset (PE/DVE balance)
                    if _pairsum(pq):
                        b2 = b2p.tile([128, R], f16, tag="b2")
                        nc.vector.tensor_tensor(
                            b2[:], gpair[:, 0, :], gpair[:, 1, :], op=Alu.add)
                        nc.tensor.matmul(d0[:], ones16[:], b2[:, 0:512],
                                         start=(pq == 0), stop=False)
                        nc.tensor.matmul(d1[:], ones16[:], b2[:, 512:1024],
                                         start=(pq == 0), stop=False)
                    else:
                        for k in range(2):
                            first = t0 + k == 0
                            last = t0 + k == TJ - 1
                            nc.tensor.matmul(d0[:], ones16[:],
                                             gpair[:, k, 0:512],
                                             start=first, stop=last)
                            nc.tensor.matmul(d1[:], ones16[:],
                                             gpair[:, k, 512:1024],
                                             start=first, stop=last)

            # bias broadcast tile (emitted late: keeps the early DVE queue
            # clear; only the epilogue consumes it)
            nc.tensor.matmul(smA[:, 256:384], ones[:], br[:],
                             start=True, stop=True)
            biasb = cp.tile([128, 128], f32)
            nc.vector.tensor_copy(biasb[:], smA[:, 256:384])
            # ---------------- epilogue ----------------
            # h[i,f] = (P^T @ W)[i,f] / den_i + bias_f, subtile-pipelined.
            with tc.tile_pool(name="ep", bufs=1) as ep:
                Pc16 = ep.tile([128, R], f16)
                nc.vector.tensor_scalar(Pc16[:, 0:512], P0[:], PSC, None,
                                        op0=Alu.mult)
                nc.vector.tensor_scalar(Pc16[:, 512:1024], P1[:], PSC, None,
                                        op0=Alu.mult)
                dsb0 = ep.tile([1, 512], f32)
                dsb1 = ep.tile([1, 512], f32)
                nc.scalar.copy(dsb0[:], d0[:])
                nc.scalar.copy(dsb1[:], d1[:])
                # den row -> column form [128, ISUB] via 1-wide transposes
                for k in range(ISUB):
                    dsb = dsb0 if k < 4 else dsb1
                    nc.tensor.matmul(
                        smB[:, 144 + k:145 + k],
                        dsb[0:1, 128 * (k % 4):128 * (k % 4) + 128],
                        ones[0:1, 0:1], start=True, stop=True)
                dcs = ep.tile([128, ISUB], f32)
                nc.vector.tensor_scalar(dcs[:], smB[:, 144:144 + ISUB], PSC,
                                        None, op0=Alu.mult)
                recb = ep.tile([128, ISUB], f32)
                nc.vector.reciprocal(recb[:], dcs[:])
                hout = ep.tile([128, ISUB, 128], f32)
                for k in range(ISUB):
                    hps = smA[:, 128 * (k % 4):128 * (k % 4) + 128]
                    nc.tensor.matmul(hps, Pc16[:, 128 * k:128 * (k + 1)],
                                     W16[:], start=True, stop=True)
                    nc.vector.scalar_tensor_tensor(
                        hout[:, k, :], hps, recb[:, k:k + 1], biasb[:],
                        op0=Alu.mult, op1=Alu.add)
                    if k % 2 == 1:
                        nc.sync.dma_start(
                            out_d.rearrange("(a p) f -> p a f", p=128)[:, k - 1:k + 1, :],
                            hout[:, k - 1:k + 1, :])

    # Walrus fits at most one sync-wait per instruction; Tile emits more.
    # Run bacc's splitter (extra waits move onto EventSemaphore insts).
    from concourse.bass import _bass_rust
    _bass_rust.generate_event_semaphores(nc)
    return nc


def kernel(adj, input, weight, bias, phi):
    """Full inputs in, full output out. Shards row-wise across 8 NeuronCores."""
    adj = np.ascontiguousarray(np.asarray(adj, dtype=np.float32))
    x = np.ascontiguousarray(np.asarray(input, dtype=np.float32))
    W = np.ascontiguousarray(np.asarray(weight, dtype=np.float32))
    b = np.ascontiguousarray(np.asarray(bias, dtype=np.float32))
    phi = np.ascontiguousarray(np.asarray(phi, dtype=np.float32))

    if not _CACHE.get("use_fallback"):
        try:
            return _kernel_bass(adj, x, W, b, phi)
        except Exception:
            import traceback
            traceback.print_exc()
            _CACHE["use_fallback"] = True
    return _kernel_jax_fallback(adj, x, W, b, phi)


def _kernel_bass(adj, x, W, b, phi):
    from concourse.bass_utils import run_bass_kernel_spmd

    if "nc" not in _CACHE:
        _CACHE["nc"] = _build_nc()
    nc = _CACHE["nc"]

    # lp[c, p, t, il] = 0 if edge/diag at (row c*R+il, col t*128+p) else -240
    # (transposed + swizzled log-space mask, f8e4m3: 0x00 / 0xF7 = -240; the
    # SWDGE accum-DMA casts to f16 while adding onto |s|)
    mask = adj.reshape(NCORES, R, TJ, 128).transpose(0, 3, 2, 1) > 0
    iloc = np.arange(R)
    for c in range(NCORES):
        gi = c * R + iloc                       # global row index
        mask[c, gi % 128, gi // 128, iloc] = True   # self-loop
    acc_tiles = [t for pq in range(TJ // 2) if pq not in DVE_PQS
                 for t in (2 * pq, 2 * pq + 1)]
    dve_tiles = [t for pq in DVE_PQS for t in (2 * pq, 2 * pq + 1)]
    lp8 = np.where(mask[:, :, acc_tiles, :],
                   np.uint8(0x00), np.uint8(0xF7))    # f8e4m3: 0 / -240
    lp16 = np.where(mask[:, :, dve_tiles, :],
                    np.float16(0), np.float16(-240))
    xT = np.ascontiguousarray(x.T).astype(np.float16)          # [128, 8192]
    x16n = np.ascontiguousarray(
        x.reshape(TJ, 128, F).transpose(1, 0, 2)).astype(np.float16)
    bp = np.ascontiguousarray(
        np.stack([b, phi[:F, 0], phi[F:, 0], np.zeros_like(b)], axis=1)
    ).astype(np.float32)

    in_maps = []
    for c in range(NCORES):
        r0 = c * R
        in_maps.append({
            "weightT": np.ascontiguousarray(W.T),
            "weight16": np.ascontiguousarray(W.astype(np.float16)),
            "biasrow": np.ascontiguousarray(b.reshape(1, F)),
            "lp8": np.ascontiguousarray(lp8[c]),
            "lp16": np.ascontiguousarray(lp16[c]),
            "x16n": x16n,
            "xT16": xT.reshape(128, TJ, 128),
            "xcT16": np.ascontiguousarray(
                xT[:, r0:r0 + R]).reshape(128, ISUB, 128),
            "bp": bp,
        })

    res = run_bass_kernel_spmd(nc, in_maps, core_ids=list(range(NCORES)),
                               trace=TRACE)
    global LAST_EXEC_NS, LAST_RESULTS
    LAST_RESULTS = res
    LAST_EXEC_NS = res.exec_time_ns
    parts = [res.results[c]["out"] for c in range(NCORES)]
    return np.concatenate(parts, axis=0).astype(np.float32)


def _kernel_jax_fallback(adj, x, W, b, phi):
    """Device fallback (sharded jax on the 8 NeuronCores) if the Bass path
    fails to compile/run in this environment."""
    import jax
    import jax.numpy as jnp
    from jax import lax
    from jax.sharding import Mesh, PartitionSpec, NamedSharding

    devs = jax.devices()[:NCORES]
    mesh = Mesh(np.asarray(devs), ("i",))
    row = NamedSharding(mesh, PartitionSpec("i", None))
    rep = NamedSharding(mesh, PartitionSpec())

    @jax.jit
    def f(adj_s, x_r, W_r, b_r, phi_r):
        xp = x_r @ W_r + b_r
        f1 = xp @ phi_r[:F]                      # [N, 1]
        f2 = xp @ phi_r[F:]                      # [N, 1]
        w = jnp.exp(jnp.float32(0.6) * f2[:, 0])  # [N]
        ri = lax.broadcasted_iota(jnp.int32, (N, N), 0)
        ci = lax.broadcasted_iota(jnp.int32, (N, N), 1)
        m = (adj_s > 0) | (ri == ci)
        G = jnp.exp(jnp.float32(0.4) * jnp.abs(f1 + f2.T))
        B = jnp.where(m, G * w[None, :], jnp.float32(0.0)).astype(jnp.float16)
        xpa = jnp.concatenate([xp, jnp.ones((N, 1), jnp.float32)],
                              axis=1).astype(jnp.float16)
        num = (B @ xpa).astype(jnp.float32)      # [N/8, F+1]
        return num[:, :F] / num[:, F:F + 1]

    args = (jax.device_put(adj, row), jax.device_put(x, rep),
            jax.device_put(W, rep), jax.device_put(b, rep),
            jax.device_put(phi, rep))
    out = f(*args)
    out.block_until_ready()
    if TRACE:
        import time
        global LAST_EXEC_NS
        reps = 5
        t0 = time.perf_counter()
        for _ in range(reps):
            out = f(*args)
        out.block_until_ready()
        LAST_EXEC_NS = int((time.perf_counter() - t0) / reps * 1e9)
    return np.asarray(out).astype(np.float32)


# revision 47
# speedup vs baseline: 1.1497x; 1.0334x over previous
"""GAT layer (nn_GAT_57543971832576) Bass/Tile kernel for 8 Trainium2 NeuronCores.

Math (reference):
    x' = x @ W + bias
    S_ij = leaky_relu(f1_i + f2_j, 0.2),  f1 = x'@phi1, f2 = x'@phi2
    A = softmax_j(where(adj+I > 0, S, -1e9))
    h = A @ x'

Reformulation used on device (core owns rows i = [c*1024, (c+1)*1024)):
    leaky_relu(s, 0.2) = 0.6*s + 0.4*|s|; softmax rows are invariant to the
    per-row shift 0.6*f1_i, so the masked attention numerator is
        B_ij = exp(0.4*|f1_i + f2_j| + 0.6*f2_j + Lp_ij)
    where Lp = 0 on edges (adj+I > 0) and -240 otherwise (additive log-space
    mask; exp underflows to exactly 0).  Using sum_j A_ij = 1 to pull the bias
    out of the values:
        P[k, i]  = sum_j x[j, k] * B^T[j, i]      (PE, lhsT = x tiles)
        den[i]   = sum_j B^T[j, i]                (PE, lhsT = ones; 24 of the
                                                   32 tile-pairs DVE-pair-summed)
        h[i, f]  = (P^T @ W)[i, f] / den_i + bias_f

    The Lp mask is folded in DURING the adjacency fetch for most tile-pairs:
    a gpsimd (SWDGE) accumulating DMA casts the fp8 mask to f16 and adds it
    onto the DVE-computed |s| tiles (CCE lines cap accumulating descriptors
    at 2048 elements, so these run at pair grain), making the per-tile
    elementwise chain just add -> abs -> (DMA +Lp) -> exp.  A few pairs
    (prologue + spread relief points) instead use a preloaded f16 mask and a
    DVE tensor add, which lets the exp stream start before the accum-DMA
    pipeline has spun up and relieves the SDMA engines mid-run.  Pairs 2-3's
    score prep is hoisted ahead of pair 0's consumers for the same reason.
    The diagonal self-loop is pre-ORed into the host-side mask, so no
    on-device correction terms are needed.

The host ships Lp pre-swizzled [p, t, i] so the device never transposes the
big matrix; the j-contraction runs directly over [j-part, i-free] tiles.
"""

import numpy as np

N = 8192
F = 128
NCORES = 8
R = N // NCORES          # rows per core (1024)
TJ = N // 128            # j-tiles (64)
ISUB = R // 128          # core-row subtiles (8)
CH = 8                   # f-projection chunks
TPC = TJ // CH           # j-tiles per chunk (8)
PSC = 0.015625           # 2^-6 epilogue rescale (f16 overflow guard)
DVE_PQS = (0, 1, 8, 14, 20, 26)  # mask via preloaded f16 + DVE add
NDVE = len(DVE_PQS)


def _pairsum(pq):
    # pairs whose den matmul is DVE pair-summed (PE/DVE balance; the very
    # last pair full-width so PE finishes right after the final exp)
    return pq % 2 == 0 or pq % 4 == 1

_CACHE = {}
TRACE = False            # set True (e.g. from test.py) to capture an NTFF profile
LAST_EXEC_NS = None      # exec time from the last traced run
LAST_RESULTS = None      # full BassKernelResults of the last run


def _build_nc():
    import concourse.bass as bass
    import concourse.mybir as mybir
    import concourse.tile as tile

    f32 = mybir.dt.float32
    f16 = mybir.dt.float16
    i16 = mybir.dt.int16
    Alu = mybir.AluOpType
    Act = mybir.ActivationFunctionType

    nc = bass.Bass("TRN2", target_bir_lowering=False, debug=False,
                   num_devices=NCORES)

    f8 = mybir.dt.float8e4
    # log-space mask: most pairs fetch fp8 via the SWDGE accum-DMA
    # (cast+add onto |s| during the load); a few pairs (the prologue,
    # before the accum stream spins up, plus two mid-run relief points)
    # use a preloaded f16 mask + DVE tensor add instead
    lp8_d = nc.dram_tensor("lp8", [128, 2 * (TJ // 2 - NDVE), R], f8,
                           kind="ExternalInput").ap()
    lp16_d = nc.dram_tensor("lp16", [128, 2 * NDVE, R], f16,
                            kind="ExternalInput").ap()
    x16n_d = nc.dram_tensor("x16n", [128, TJ, F], f16, kind="ExternalInput").ap()
    xT16_d = nc.dram_tensor("xT16", [128, TJ, 128], f16, kind="ExternalInput").ap()
    xcT_d = nc.dram_tensor("xcT16", [128, ISUB, 128], f16, kind="ExternalInput").ap()
    W16_d = nc.dram_tensor("weight16", [128, 128], f16, kind="ExternalInput").ap()
    WT_d = nc.dram_tensor("weightT", [128, 128], f32, kind="ExternalInput").ap()
    bp_d = nc.dram_tensor("bp", [128, 4], f32, kind="ExternalInput").ap()
    br_d = nc.dram_tensor("biasrow", [1, 128], f32, kind="ExternalInput").ap()
    out_d = nc.dram_tensor("out", [R, F], f32, kind="ExternalOutput").ap()

    with tile.TileContext(nc) as tc:
        with tc.tile_pool(name="const", bufs=1) as cp, \
             tc.tile_pool(name="mmps", bufs=1, space="PSUM") as mmps, \
             tc.tile_pool(name="ppsA", bufs=1, space="PSUM") as ppsA, \
             tc.tile_pool(name="eps", bufs=2, space="PSUM") as eps:
            # ---------------- psum layout (8 banks) ----------------
            P0 = mmps.tile([128, 512], f32, name="P0")
            P1 = mmps.tile([128, 512], f32, name="P1")
            d0 = mmps.tile([1, 512], f32, name="d0")
            d1 = mmps.tile([1, 512], f32, name="d1")
            smA = ppsA.tile([128, 512], f32, name="smA")   # prep scratch
            smB = ppsA.tile([128, 160], f32, name="smB")   # f-projections + den col

            # ---------------- constants + inputs ----------------
            ones = cp.tile([1, 128], f32)
            nc.vector.memset(ones[:], 1.0)
            ones16 = cp.tile([128, 1], f16)
            nc.vector.memset(ones16[:], 1.0)
            ones16r = cp.tile([1, 128], f16)
            nc.vector.memset(ones16r[:], 1.0)
            # pre-warm the exp table load and the SWDGE (Q7) DMA path so
            # neither first-use cost lands on the first real exp / mask DMA
            warm = cp.tile([1, 128], f16)
            nc.scalar.activation(warm[0:1, 0:8], ones16r[0:1, 0:8],
                                 Act.Exp, scale=0.0)

            WT = cp.tile([128, 128], f32)
            nc.sync.dma_start(WT[:], WT_d)
            bp = cp.tile([128, 4], f32)
            nc.sync.dma_start(bp[:], bp_d)
            xcT = cp.tile([128, ISUB, 128], f16)
            nc.sync.dma_start(xcT[:], xcT_d)
            xT16 = cp.tile([128, TJ, 128], f16)
            x16 = cp.tile([128, TJ, F], f16)
            br = cp.tile([1, 128], f32)
            W16 = cp.tile([128, 128], f16)

            # ---------------- prep ----------------
            # Only WT/bp/xcT are in flight here: every DMA emitted before a
            # compute op can end up on its wait path (pooled completion
            # lanes), so the big x loads are emitted after the f1b prep.
            # Wphi = W @ phi (host supplies W^T), so f = x @ Wphi
            nc.tensor.matmul(smA[:, 128:130], WT[:], bp[:, 1:3],
                             start=True, stop=True)
            Wphi16 = cp.tile([128, 2], f16)
            nc.vector.tensor_copy(Wphi16[:], smA[:, 128:130])

            # core-row f1 projections: f1row = phi1^T @ Xcore^T (f16 rhs so
            # the broadcast matmul runs at f16 rate; c12 is folded into f2c)
            f1row = cp.tile([1, R], f16)
            f1b = cp.tile([128, R], f16)
            for g in range(2):
                fr = eps.tile([128, 512], f32, tag="fps")
                nc.tensor.matmul(
                    fr[0:1, :], Wphi16[:, 0:1],
                    xcT[:, 4 * g:4 * g + 4, :].rearrange("p a b -> p (a b)"),
                    start=True, stop=True)
                if g == 0:
                    nc.scalar.copy(f1row[0:1, 0:512], fr[0:1, :])
                else:
                    nc.vector.tensor_copy(f1row[0:1, 512:1024], fr[0:1, :])
            for g in range(2):
                fb = eps.tile([128, 512], f32, tag="fps")
                nc.tensor.matmul(fb[:], ones16r[:],
                                 f1row[0:1, 512 * g:512 * (g + 1)],
                                 start=True, stop=True)
                nc.vector.tensor_copy(f1b[:, 512 * g:512 * (g + 1)], fb[:])

            # big x loads + masks, emitted only now (see note above)
            nc.sync.dma_start(xT16[:, 0:TPC, :], xT16_d[:, 0:TPC, :])
            nc.sync.dma_start(x16[:, 0:2 * TPC, :], x16n_d[:, 0:2 * TPC, :])
            nc.sync.dma_start(xT16[:, TPC:3 * TPC, :], xT16_d[:, TPC:3 * TPC, :])
            nc.sync.dma_start(br[:], br_d)
            nc.sync.dma_start(W16[:], W16_d)
            # c1 = b@phi1, c2 = b@phi2 (bias-fold constants), broadcasts
            nc.tensor.matmul(smA[0:1, 132:134], bp[:, 0:1], bp[:, 1:3],
                             start=True, stop=True)
            crow = cp.tile([1, 2], f32)
            nc.scalar.copy(crow[:], smA[0:1, 132:134])
            c12 = cp.tile([1, 1], f32)
            nc.vector.tensor_tensor(c12[:], crow[0:1, 0:1], crow[0:1, 1:2],
                                    op=Alu.add)
            nc.tensor.matmul(smA[:, 136:137], ones[:], c12[:],
                             start=True, stop=True)
            c12b = cp.tile([128, 1], f32)
            nc.scalar.copy(c12b[:], smA[:, 136:137])
            nc.tensor.matmul(smA[:, 140:141], ones[:], crow[0:1, 1:2],
                             start=True, stop=True)
            c2b06 = cp.tile([128, 1], f32)
            nc.scalar.activation(c2b06[:], smA[:, 140:141], Act.Copy, scale=0.6)


            f2c = cp.tile([128, TJ], f32)
            wb = cp.tile([128, TJ], f32)    # ACT bias: 0.6*f2_j (= log w_j)

            def emit_fcols(ch):
                # chunk ch's f2 projections (column form) + exp-bias scalars
                c0 = 16 * ch
                for tt in range(TPC):
                    t = ch * TPC + tt
                    nc.tensor.matmul(smB[:, 2 * t:2 * t + 2],
                                     xT16[:, t, :], Wphi16[:],
                                     start=True, stop=True)
                nc.vector.tensor_scalar(f2c[:, ch * TPC:(ch + 1) * TPC],
                                        smB[:, c0 + 1:c0 + 16:2],
                                        c12b[:], None, op0=Alu.add)
                nc.vector.tensor_scalar(wb[:, ch * TPC:(ch + 1) * TPC],
                                        smB[:, c0 + 1:c0 + 16:2],
                                        0.6, c2b06[:],
                                        op0=Alu.mult, op1=Alu.add)

            # ---------------- main loop (32 tile-pairs) ----------------
            with tc.tile_pool(name="sp", bufs=8) as spp, \
                 tc.tile_pool(name="gp", bufs=4) as gpp, \
                 tc.tile_pool(name="lpp", bufs=3) as lpp, \
                 tc.tile_pool(name="b2p", bufs=2) as b2p:
                emit_fcols(0)
                emit_fcols(1)
                # trailing big x slices: second HWDGE ring (ACT queue, idle
                # this early), and emitted AFTER the prep section -- DMA
                # completion semaphores are pooled into 8 lanes, so anything
                # emitted before the prep matmuls would be transitively
                # waited on by them (costs ~15us of prologue)
                nc.scalar.dma_start(xT16[:, 3 * TPC:TJ, :],
                                    xT16_d[:, 3 * TPC:TJ, :])
                nc.scalar.dma_start(x16[:, 2 * TPC:TJ, :],
                                    x16n_d[:, 2 * TPC:TJ, :])
                # f16 masks for the DVE-add pairs, loaded up front (small)
                lp16 = cp.tile([128, 2 * NDVE, R], f16)
                nc.sync.dma_start(lp16[:], lp16_d[:])

                sptiles = {}

                def emit_score(pq):
                    # |s| tiles + mask for pair pq (DVE-add for the first
                    # two pairs; SWDGE accum-DMA otherwise)
                    t0 = 2 * pq
                    spair = spp.tile([128, 2, R], f16, tag="sp")
                    sptiles[pq] = spair
                    for k in range(2):
                        nc.vector.tensor_scalar(
                            spair[:, k, :], f1b[:],
                            f2c[:, t0 + k:t0 + k + 1], None, op0=Alu.add)
                    nc.vector.tensor_scalar(
                        spair[:].rearrange("p a b -> p (a b)").bitcast(i16),
                        spair[:].rearrange("p a b -> p (a b)").bitcast(i16),
                        0x7FFF, None, op0=Alu.bitwise_and)
                    if pq in DVE_PQS:
                        sl = 2 * DVE_PQS.index(pq)
                        nc.vector.tensor_tensor(
                            spair[:].rearrange("p a b -> p (a b)"),
                            spair[:].rearrange("p a b -> p (a b)"),
                            lp16[:, sl:sl + 2, :].rearrange("p a b -> p (a b)"),
                            op=Alu.add)
                    else:
                        na = pq - sum(1 for q in DVE_PQS if q < pq)
                        nc.gpsimd.dma_start(spair[:],
                                            lp8_d[:, 2 * na:2 * na + 2, :],
                                            accum_op=Alu.add)

                # hoist the first four pairs' score prep: pair 0 first so
                # exp0 fires as early as possible, then 2-3 so the accum-DMA
                # pipeline spins up under the first DVE exps
                emit_score(0)
                emit_score(1)
                emit_score(2)
                emit_score(3)
                for pq in range(TJ // 2):
                    t0 = 2 * pq
                    if t0 % TPC == 0 and t0 // TPC + 2 < CH:
                        emit_fcols(t0 // TPC + 2)
                    if pq in sptiles:
                        spair = sptiles.pop(pq)
                    else:
                        emit_score(pq)
                        spair = sptiles.pop(pq)
                    gpair = gpp.tile([128, 2, R], f16, tag="g")
                    for k in range(2):
                        nc.scalar.activation(
                            gpair[:, k, :], spair[:, k, :], Act.Exp,
                            scale=0.4, bias=wb[:, t0 + k:t0 + k + 1])
                    for k in range(2):
                        t = t0 + k
                        nc.tensor.matmul(P0[:], x16[:, t, :],
                                         gpair[:, k, 0:512],
                                         start=(t == 0), stop=(t == TJ - 1))
                        nc.tensor.matmul(P1[:], x16[:, t, :],
                                         gpair[:, k, 512:1024],
                                         start=(t == 0), stop=(t == TJ - 1))
                    # den: DVE pair-sum for a spread subset (PE/DVE balance)
                    if _pairsum(pq):
                        b2 = b2p.tile([128, R], f16, tag="b2")
                        nc.vector.tensor_tensor(
                            b2[:], gpair[:, 0, :], gpair[:, 1, :], op=Alu.add)
                        nc.tensor.matmul(d0[:], ones16[:], b2[:, 0:512],
                                         start=(pq == 0), stop=False)
                        nc.tensor.matmul(d1[:], ones16[:], b2[:, 512:1024],
                                         start=(pq == 0), stop=False)
                    else:
                        for k in range(2):
                            first = t0 + k == 0
                            last = t0 + k == TJ - 1
                            nc.tensor.matmul(d0[:], ones16[:],
                                             gpair[:, k, 0:512],
                                             start=first, stop=last)
                            nc.tensor.matmul(d1[:], ones16[:],
                                             gpair[:, k, 512:1024],
                                             start=first, stop=last)

            # bias broadcast tile (emitted late: keeps the early DVE queue
            # clear; only the epilogue consumes it)
            nc.tensor.matmul(smA[:, 256:384], ones[:], br[:],
                             start=True, stop=True)
            biasb = cp.tile([128, 128], f32)
            nc.vector.tensor_copy(biasb[:], smA[:, 256:384])
            # ---------------- epilogue ----------------
            # h[i,f] = (P^T @ W)[i,f] / den_i + bias_f, subtile-pipelined.
            with tc.tile_pool(name="ep", bufs=1) as ep:
                Pc16 = ep.tile([128, R], f16)
                nc.vector.tensor_scalar(Pc16[:, 0:512], P0[:], PSC, None,
                                        op0=Alu.mult)
                nc.vector.tensor_scalar(Pc16[:, 512:1024], P1[:], PSC, None,
                                        op0=Alu.mult)
                dsb0 = ep.tile([1, 512], f32)
                dsb1 = ep.tile([1, 512], f32)
                nc.scalar.copy(dsb0[:], d0[:])
                nc.scalar.copy(dsb1[:], d1[:])
                # den row -> column form [128, ISUB] via 1-wide transposes
                for k in range(ISUB):
                    dsb = dsb0 if k < 4 else dsb1
                    nc.tensor.matmul(
                        smB[:, 144 + k:145 + k],
                        dsb[0:1, 128 * (k % 4):128 * (k % 4) + 128],
                        ones[0:1, 0:1], start=True, stop=True)
                dcs = ep.tile([128, ISUB], f32)
                nc.vector.tensor_scalar(dcs[:], smB[:, 144:144 + ISUB], PSC,
                                        None, op0=Alu.mult)
                recb = ep.tile([128, ISUB], f32)
                nc.vector.reciprocal(recb[:], dcs[:])
                hout = ep.tile([128, ISUB, 128], f32)
                for k in range(ISUB):
                    hps = smA[:, 128 * (k % 4):128 * (k % 4) + 128]
                    nc.tensor.matmul(hps, Pc16[:, 128 * k:128 * (k + 1)],
                                     W16[:], start=True, stop=True)
                    nc.vector.scalar_tensor_tensor(
                        hout[:, k, :], hps, recb[:, k:k + 1], biasb[:],
                        op0=Alu.mult, op1=Alu.add)
                    if k % 2 == 1:
                        nc.sync.dma_start(
                            out_d.rearrange("(a p) f -> p a f", p=128)[:, k - 1:k + 1, :],
                            hout[:, k - 1:k + 1, :])

    # Walrus fits at most one sync-wait per instruction; Tile emits more.
    # Run bacc's splitter (extra waits move onto EventSemaphore insts).
    from concourse.bass import _bass_rust
    _bass_rust.generate_event_semaphores(nc)
    return nc


def kernel(adj, input, weight, bias, phi):
    """Full inputs in, full output out. Shards row-wise across 8 NeuronCores."""
    adj = np.ascontiguousarray(np.asarray(adj, dtype=np.float32))
    x = np.ascontiguousarray(np.asarray(input, dtype=np.float32))
    W = np.ascontiguousarray(np.asarray(weight, dtype=np.float32))
    b = np.ascontiguousarray(np.asarray(bias, dtype=np.float32))
    phi = np.ascontiguousarray(np.asarray(phi, dtype=np.float32))

    if not _CACHE.get("use_fallback"):
        try:
            return _kernel_bass(adj, x, W, b, phi)
        except Exception:
            import traceback
            traceback.print_exc()
            _CACHE["use_fallback"] = True
    return _kernel_jax_fallback(adj, x, W, b, phi)


def _kernel_bass(adj, x, W, b, phi):
    from concourse.bass_utils import run_bass_kernel_spmd

    if "nc" not in _CACHE:
        _CACHE["nc"] = _build_nc()
    nc = _CACHE["nc"]

    # lp[c, p, t, il] = 0 if edge/diag at (row c*R+il, col t*128+p) else -240
    # (transposed + swizzled log-space mask, f8e4m3: 0x00 / 0xF7 = -240; the
    # SWDGE accum-DMA casts to f16 while adding onto |s|)
    mask = adj.reshape(NCORES, R, TJ, 128).transpose(0, 3, 2, 1) > 0
    iloc = np.arange(R)
    for c in range(NCORES):
        gi = c * R + iloc                       # global row index
        mask[c, gi % 128, gi // 128, iloc] = True   # self-loop
    acc_tiles = [t for pq in range(TJ // 2) if pq not in DVE_PQS
                 for t in (2 * pq, 2 * pq + 1)]
    dve_tiles = [t for pq in DVE_PQS for t in (2 * pq, 2 * pq + 1)]
    lp8 = np.where(mask[:, :, acc_tiles, :],
                   np.uint8(0x00), np.uint8(0xF7))    # f8e4m3: 0 / -240
    lp16 = np.where(mask[:, :, dve_tiles, :],
                    np.float16(0), np.float16(-240))
    xT = np.ascontiguousarray(x.T).astype(np.float16)          # [128, 8192]
    x16n = np.ascontiguousarray(
        x.reshape(TJ, 128, F).transpose(1, 0, 2)).astype(np.float16)
    bp = np.ascontiguousarray(
        np.stack([b, phi[:F, 0], phi[F:, 0], np.zeros_like(b)], axis=1)
    ).astype(np.float32)

    in_maps = []
    for c in range(NCORES):
        r0 = c * R
        in_maps.append({
            "weightT": np.ascontiguousarray(W.T),
            "weight16": np.ascontiguousarray(W.astype(np.float16)),
            "biasrow": np.ascontiguousarray(b.reshape(1, F)),
            "lp8": np.ascontiguousarray(lp8[c]),
            "lp16": np.ascontiguousarray(lp16[c]),
            "x16n": x16n,
            "xT16": xT.reshape(128, TJ, 128),
            "xcT16": np.ascontiguousarray(
                xT[:, r0:r0 + R]).reshape(128, ISUB, 128),
            "bp": bp,
        })

    res = run_bass_kernel_spmd(nc, in_maps, core_ids=list(range(NCORES)),
                               trace=TRACE)
    global LAST_EXEC_NS, LAST_RESULTS
    LAST_RESULTS = res
    LAST_EXEC_NS = res.exec_time_ns
    parts = [res.results[c]["out"] for c in range(NCORES)]
    return np.concatenate(parts, axis=0).astype(np.float32)


def _kernel_jax_fallback(adj, x, W, b, phi):
    """Device fallback (sharded jax on the 8 NeuronCores) if the Bass path
    fails to compile/run in this environment."""
    import jax
    import jax.numpy as jnp
    from jax import lax
    from jax.sharding import Mesh, PartitionSpec, NamedSharding

    devs = jax.devices()[:NCORES]
    mesh = Mesh(np.asarray(devs), ("i",))
    row = NamedSharding(mesh, PartitionSpec("i", None))
    rep = NamedSharding(mesh, PartitionSpec())

    @jax.jit
    def f(adj_s, x_r, W_r, b_r, phi_r):
        xp = x_r @ W_r + b_r
        f1 = xp @ phi_r[:F]                      # [N, 1]
        f2 = xp @ phi_r[F:]                      # [N, 1]
        w = jnp.exp(jnp.float32(0.6) * f2[:, 0])  # [N]
        ri = lax.broadcasted_iota(jnp.int32, (N, N), 0)
        ci = lax.broadcasted_iota(jnp.int32, (N, N), 1)
        m = (adj_s > 0) | (ri == ci)
        G = jnp.exp(jnp.float32(0.4) * jnp.abs(f1 + f2.T))
        B = jnp.where(m, G * w[None, :], jnp.float32(0.0)).astype(jnp.float16)
        xpa = jnp.concatenate([xp, jnp.ones((N, 1), jnp.float32)],
                              axis=1).astype(jnp.float16)
        num = (B @ xpa).astype(jnp.float32)      # [N/8, F+1]
        return num[:, :F] / num[:, F:F + 1]

    args = (jax.device_put(adj, row), jax.device_put(x, rep),
            jax.device_put(W, rep), jax.device_put(b, rep),
            jax.device_put(phi, rep))
    out = f(*args)
    out.block_until_ready()
    if TRACE:
        import time
        global LAST_EXEC_NS
        reps = 5
        t0 = time.perf_counter()
        for _ in range(reps):
            out = f(*args)
        out.block_until_ready()
        LAST_EXEC_NS = int((time.perf_counter() - t0) / reps * 1e9)
    return np.asarray(out).astype(np.float32)
